# revision 22
# baseline (speedup 1.0000x reference)
"""FFD sparse-matmul kernel for Trainium2 (8 NeuronCores).

Problem: out[b, r, d] = sum_i 1[rows_i == r] * vals_i * (x[b, cols_i, d]*scale[d] - offset[d])
  = (A @ xs)[r, j] with xs[k, j=b*3+d] = x[b, k, d]*scale[d] - offset[d]
where A is the static [200000, 4096] sparse FFD matrix (12.8M nnz).

Strategy: densify A on the host into per-row-scaled fp8 and stream it
through the TensorEngine as the 512-wide MOVING operand; the tiny
control-point matrix xs is the stationary operand (fp8 hi + lo split so
xs quantization error is negligible; combined on the host as
hi + lo/16). This keeps the PE at its streaming roofline instead of the
LDWEIGHTS/SWDGE-bound v1 layout, and halves HBM traffic vs fp16 weights
(103 MB/core ~ 287 us at the 358 GB/s per-NC HBM floor) with plain
HWDGE DMA on both rings (no SWDGE cast - the 552-us bottleneck of v1).
psum[*, 512] accumulates across the contraction; ACT copies psum->SBUF;
host applies per-row/per-column scales.

MODE:
  "e4dr": fp8e4 (e4m3) weights with MatmulPerfMode.DoubleRow - 2 fp8
      MACs/cell/cycle, K=256 per matmul -> PE ~190 us, DMA-bound ~300 us.
      Output rel err ~1.2e-2 (harness gate 2e-2).
  "e3":   fp8e3 (e3m4) weights, normal mode - PE ~380 us, rel err ~6e-3.
"""

import os
import numpy as np
import ml_dtypes

MODE = "e4dr"

N_PTS = 200000
N_CTRL = 4096
B = 2
N_CORES = 8
ROWS_PER_CORE = N_PTS // N_CORES  # 25000
FD = 512                          # moving free dim per matmul (= 1 PSUM bank)
N_TILES = -(-ROWS_PER_CORE // FD)  # 49
R_PAD = N_TILES * FD              # 25088
KC = 128                          # partition (contraction) dim
FN = B * 3                        # 6 logical output columns (j = b*3 + d)
LO_SCALE = 16.0                   # xs residual scale for the lo fp8 half

if MODE == "e4dr":
    F8NP, F8MAX = ml_dtypes.float8_e4m3, 240.0
    N_SUP = N_CTRL // (2 * KC)    # 16 DoubleRow superchunks (K=256 each)
    SC = 16                       # stationary cols: hi(6) | lo(6) | pad(4)
    # Rows 25000..25087 are padding; the last tile only needs 424 rows.
    # 432 keeps the moving AP's inner step a multiple of 16 (DoubleRow
    # constraint) while skipping 80 rows of zero-byte DMA + compute.
    LAST_FD = 432
else:
    F8NP, F8MAX = ml_dtypes.float8_e3m4, 15.5
    N_CHUNKS = N_CTRL // KC       # 32
    SC = 12                       # stationary cols: hi(6) | lo(6)

LAST_RESULTS = None  # BassKernelResults of the most recent device run

_static_cache = {}  # fingerprint -> (wT per core, row scales per core)
_nc_cache = {}


def _fingerprint(*arrays):
    h = 0
    for a in arrays:
        s = a[:: max(1, a.size // 4096)].tobytes()
        h ^= hash((a.size, s, float(a.astype(np.float64).sum())))
    return h


def _install_profile_shim():
    """Make trace=True work in images whose antenv lacks axon_hooks, and
    neuter the bucket artifact upload. Best-effort; harmless if partial."""
    import sys
    import types

    try:
        import concourse.bass_utils as bu

        bu.upload_artifacts = lambda tmpdir: f"local:{tmpdir}"
    except Exception:
        pass
    try:
        import antenv.axon_hooks  # noqa: F401

        return
    except ImportError:
        pass
    try:
        mod = types.ModuleType("antenv.axon_hooks")
        mod._hook = None
        mod.set_axon_ntff_profile_hook = lambda h: setattr(mod, "_hook", h)
        mod.get_axon_ntff_profile_hook = lambda: mod._hook
        sys.modules["antenv.axon_hooks"] = mod
        import antenv

        antenv.axon_hooks = mod
        if "/root/.axon_site/trn_agent_boot" not in sys.path:
            sys.path.insert(0, "/root/.axon_site/trn_agent_boot")
        from trn_boot import _ntff_profile_via_ctypes

        hook = _ntff_profile_via_ctypes("/opt/axon/libaxon_pjrt.so")
        if hook is not None:
            mod._hook = hook
    except Exception:
        pass


def _build_nc():
    import concourse.mybir as mybir
    from concourse import bacc
    from concourse.tile import TileContext

    f32 = mybir.dt.float32
    f8 = mybir.dt.float8e4 if MODE == "e4dr" else mybir.dt.float8e3
    nc = bacc.Bacc()
    if MODE == "e4dr":
        wT = nc.declare_dram_parameter(
            "wT", [N_TILES - 1, KC, N_SUP, 2, FD], f8, isOutput=False
        )
        wLast = nc.declare_dram_parameter(
            "wLast", [KC, N_SUP, 2, LAST_FD], f8, isOutput=False
        )
        xs = nc.declare_dram_parameter("xs", [KC, N_SUP, 2, SC], f8, isOutput=False)
    else:
        wT = nc.declare_dram_parameter(
            "wT", [N_TILES, KC, N_CHUNKS * FD], f8, isOutput=False
        )
        xs = nc.declare_dram_parameter("xs", [KC, N_CHUNKS * SC], f8, isOutput=False)
    out = nc.declare_dram_parameter("out", [SC, N_TILES * FD], f32, isOutput=True)

    # Two HWDGE rings (qSPDynamicHW via nc.sync, qActDynamicHW via
    # nc.scalar); alternate weight tiles between them (~322 GB/s aggregate
    # vs one ring's ~292). Routing tiles through the SWDGE path (nc.gpsimd)
    # was measured both slower AND incorrect for fp8 - do not use it.
    queues = [nc.sync, nc.scalar]
    wshape = [KC, N_SUP, 2, FD] if MODE == "e4dr" else [KC, N_CHUNKS * FD]

    def mm(ps, xs_sb, w_sb, i, last):
        if MODE == "e4dr":
            nc.tensor.matmul(
                ps[:],
                xs_sb[:, i],
                w_sb[:, i],
                start=(i == 0),
                stop=(i == last),
                perf_mode=mybir.MatmulPerfMode.DoubleRow,
            )
        else:
            nc.tensor.matmul(
                ps[:],
                xs_sb[:, i * SC : (i + 1) * SC],
                w_sb[:, i * FD : (i + 1) * FD],
                start=(i == 0),
                stop=(i == last),
            )

    n_inner = N_SUP if MODE == "e4dr" else N_CHUNKS

    with TileContext(nc) as tc:
        with (
            tc.tile_pool(name="wp", bufs=6) as wp,
            tc.tile_pool(name="cp", bufs=1) as cp,
            tc.tile_pool(name="pp", bufs=4, space="PSUM") as pp,
        ):
            # Tile 0 arrives in 8 sub-DMAs (split across both rings) so the
            # PE can start after ~1/8 of the tile instead of the full 2 MB.
            # (Splitting EVERY tile was measured slower - per-DMA completion
            # overhead outweighs the overlap once the pipeline is warm.)
            w_first = wp.tile(wshape, f8, tag="w")
            if MODE == "e4dr":
                qs = N_SUP // 8
                for i in range(8):
                    queues[i % 2].dma_start(
                        out=w_first[:, i * qs : (i + 1) * qs],
                        in_=wT[0][:, i * qs : (i + 1) * qs],
                    )
            else:
                q = N_CHUNKS * FD // 8
                for i in range(8):
                    queues[i % 2].dma_start(
                        out=w_first[:, i * q : (i + 1) * q],
                        in_=wT[0][:, i * q : (i + 1) * q],
                    )
            xs_sb = cp.tile(
                [KC, N_SUP, 2, SC] if MODE == "e4dr" else [KC, N_CHUNKS * SC],
                f8,
                tag="xs",
            )
            nc.scalar.dma_start(out=xs_sb[:], in_=xs[:])
            obuf = cp.tile([SC, N_TILES * FD], f32, tag="obuf")
            early = (N_TILES - 1) * FD
            for t in range(N_TILES):
                fd = FD
                if t == 0:
                    w_sb = w_first
                elif MODE == "e4dr" and t == N_TILES - 1:
                    # Last tile arrives in 4 sub-DMAs so its compute hides
                    # inside its own DMA window (no steady-state cost here,
                    # unlike splitting every tile).
                    fd = LAST_FD
                    w_sb = wp.tile([KC, N_SUP, 2, LAST_FD], f8, tag="w")
                    qq = N_SUP // 4
                    for i in range(4):
                        queues[(t + i) % 2].dma_start(
                            out=w_sb[:, i * qq : (i + 1) * qq],
                            in_=wLast[:, i * qq : (i + 1) * qq],
                        )
                else:
                    w_sb = wp.tile(wshape, f8, tag="w")
                    queues[t % 2].dma_start(out=w_sb[:], in_=wT[t])
                if t == N_TILES - 1:
                    # Most of the output is final by now; stream it out while
                    # the last tile computes so only ~2 tiles' worth remains.
                    nc.sync.dma_start(out=out[:, :early], in_=obuf[:, :early])
                ps = pp.tile([SC, fd], f32)
                for i in range(n_inner):
                    mm(ps, xs_sb, w_sb, i, n_inner - 1)
                nc.scalar.copy(out=obuf[:, t * FD : t * FD + fd], in_=ps[:])
            nc.scalar.dma_start(out=out[:, early:], in_=obuf[:, early:])
    nc.finalize()
    return nc


def _prepare_static(ffd_vals, ffd_rows, ffd_cols):
    """Densify + quantize the static sparse matrix into per-core fp8
    moving-operand tiles.

    e3:   wT[t, p, kc*FD + c]    = q(A[t*FD+c, kc*KC + p])
    e4dr: wT[t, p, sc, i, c]     = q(A[t*FD+c, sc*256 + i*128 + p])
    """
    key = (_fingerprint(ffd_vals, ffd_rows, ffd_cols), MODE)
    if key in _static_cache:
        return _static_cache[key]

    try:
        from scipy.sparse import coo_matrix

        A = np.asarray(
            coo_matrix(
                (ffd_vals, (ffd_rows, ffd_cols)), shape=(N_PTS, N_CTRL)
            ).todense(),
            dtype=np.float32,
        )
    except Exception:
        A = np.zeros((N_PTS, N_CTRL), np.float32)
        np.add.at(A, (ffd_rows, ffd_cols), ffd_vals)

    wTs, rscales = [], []
    for c in range(N_CORES):
        Ac = A[c * ROWS_PER_CORE : (c + 1) * ROWS_PER_CORE]
        rowmax = np.maximum(Ac.max(axis=1), 1e-30).astype(np.float32)
        s = rowmax / F8MAX
        Ap = np.zeros((R_PAD, N_CTRL), F8NP)
        Ap[:ROWS_PER_CORE] = (Ac / s[:, None]).astype(F8NP)
        s_pad = np.ones(R_PAD, np.float32)
        s_pad[:ROWS_PER_CORE] = s
        if MODE == "e4dr":
            # [r, k] -> [t, p, sc, i, c]: r = t*FD + c, k = sc*256 + i*128 + p
            w = Ap.reshape(N_TILES, FD, N_SUP, 2, KC).transpose(0, 4, 2, 3, 1)
            wTs.append(np.ascontiguousarray(w))
        else:
            # [r, k] -> [t, p, kc, c]: r = t*FD + c, k = kc*KC + p
            w = Ap.reshape(N_TILES, FD, N_CHUNKS, KC).transpose(0, 3, 2, 1)
            wTs.append(np.ascontiguousarray(w).reshape(N_TILES, KC, N_CHUNKS * FD))
        rscales.append(s_pad)

    _static_cache.clear()
    _static_cache[key] = (wTs, rscales)
    return wTs, rscales


def kernel(x, scale_vec, offset, ffd_vals, ffd_rows, ffd_cols):
    global LAST_RESULTS
    from concourse.bass_utils import run_bass_kernel_spmd

    x = np.asarray(x, np.float32)
    scale_vec = np.asarray(scale_vec, np.float32)
    offset = np.asarray(offset, np.float32)
    ffd_vals = np.asarray(ffd_vals, np.float32)
    ffd_rows = np.asarray(ffd_rows, np.int32)
    ffd_cols = np.asarray(ffd_cols, np.int32)

    wTs, rscales = _prepare_static(ffd_vals, ffd_rows, ffd_cols)

    # Dynamic (per-call) host prep: tiny.
    # xs[k, j=b*3+d] = x[b,k,d]*scale[d] - offset[d]
    xs6 = (
        x * scale_vec[None, None, :] - offset[None, None, :]
    ).transpose(1, 0, 2).reshape(N_CTRL, FN).astype(np.float32)
    t_j = np.maximum(np.abs(xs6).max(axis=0), 1e-30).astype(np.float32) / F8MAX
    xn = xs6 / t_j[None, :]
    hi = xn.astype(F8NP)
    lo = np.clip(
        (xn - hi.astype(np.float32)) * LO_SCALE, -F8MAX, F8MAX
    ).astype(F8NP)
    stat = np.zeros((N_CTRL, SC), F8NP)
    stat[:, :FN] = hi
    stat[:, FN : 2 * FN] = lo
    if MODE == "e4dr":
        # [k, j] -> [p, sc, i, j]: k = sc*256 + i*128 + p
        stat = np.ascontiguousarray(
            stat.reshape(N_SUP, 2, KC, SC).transpose(2, 0, 1, 3)
        )
    else:
        stat = np.ascontiguousarray(
            stat.reshape(N_CHUNKS, KC, SC).transpose(1, 0, 2).reshape(
                KC, N_CHUNKS * SC
            )
        )

    if MODE == "e4dr":
        in_maps = [
            {
                "wT": wTs[c][: N_TILES - 1],
                "wLast": np.ascontiguousarray(wTs[c][-1][:, :, :, :LAST_FD]),
                "xs": stat,
            }
            for c in range(N_CORES)
        ]
    else:
        in_maps = [{"wT": wTs[c], "xs": stat} for c in range(N_CORES)]

    if MODE not in _nc_cache:
        _nc_cache[MODE] = _build_nc()
    nc = _nc_cache[MODE]

    trace = bool(os.environ.get("BASS_TRACE"))
    if trace:
        _install_profile_shim()
    try:
        res = run_bass_kernel_spmd(nc, in_maps, list(range(N_CORES)), trace=trace)
    except Exception:
        if not trace:
            raise
        os.environ.pop("BASS_TRACE", None)
        res = run_bass_kernel_spmd(nc, in_maps, list(range(N_CORES)), trace=False)
    LAST_RESULTS = res

    shards = []
    for c in range(N_CORES):
        o = np.asarray(res.results[c]["out"], np.float32)  # [SC, R_PAD]
        comb = o[:FN] + o[FN : 2 * FN] / LO_SCALE          # [FN, R_PAD]
        o6 = comb.T[:ROWS_PER_CORE] * (
            rscales[c][:ROWS_PER_CORE, None] * t_j[None, :]
        )
        shards.append(o6)
    full6 = np.concatenate(shards, axis=0)  # [N_PTS, FN]
    out = np.ascontiguousarray(
        full6.reshape(N_PTS, B, 3).transpose(1, 0, 2)
    ).astype(np.float32)
    return out
